# revision 12
# baseline (speedup 1.0000x reference)
"""GNN NodeModel kernel for 8 Trainium2 NeuronCores (Bass/Tile).

Full-input contract: kernel(**inputs) takes the unsharded numpy inputs and
returns the full [N, D] output. Internally:
  - edges are sorted by destination node; each core owns N/8 nodes plus all
    edges targeting them
  - phase A (edge-parallel): gather x[row] / edge_attr via indirect DMA,
    g = relu([xg, ea] @ W1a + b1a)  -> DRAM scratch (per-core)
  - phase B (node-parallel): per 128-node tile, gather the tile's edges' g
    rows, segment-sum via a selection-matrix matmul, then node MLP2 with
    host-folded weights:
      out = relu(x@B1 + rmean@(W1b@B2) + b2a + nonempty*(b1b@B2)) @ W2b + b2b
    where [B1; B2] = W2a.  (Folds the edge-side W1b matmul into the node
    side; exact for non-empty nodes, the nonempty mask handles the rest.)
  - nodes are bin-packed into 128-node tiles balancing edge counts (keeps the
    per-tile gather padding F2 minimal); x is pre-transposed on the host and
    the output is produced transposed (saves all x/out on-chip transposes)

All matmuls run in float32r (full PE rate at free dim >= 256). Tensors that
only feed matmuls are staged as f32r externals (binding rounds them to the
same grid the PE uses anyway); x/edge_attr gathers cast f32->f32r in the DMA.
"""

import sys

sys.path.insert(0, "/opt/trn_rl_repo")

import heapq
from contextlib import ExitStack

import numpy as np

import concourse.bass as bass
import concourse.tile as tile
from concourse import bacc, mybir
from concourse.bass_utils import run_bass_kernel_spmd

N = 20000
E = 80000
D = 1024
C = 8           # cores
NPC = N // C    # nodes per core (2500)
NP = 2560       # padded node slots per core (20 x 128)
NSEG = NP // 128          # 20 segment tiles of 128 node slots
NT2 = NP // 256           # 10 MLP2 tiles of 256 node slots
F32 = mybir.dt.float32
F32R = mybir.dt.float32r
I32 = mybir.dt.int32

AF = mybir.ActivationFunctionType
OP = mybir.AluOpType

_PROGRAM_CACHE = {}
_LAST_IN_MAPS = None


def _build_program(EC, F2, reps=1):
    """Build the SPMD Bass program. EC = edge slot capacity per core
    (multiple of 256); F2 = 128-edge gather subtiles per 128-node tile.
    reps > 1 repeats the whole pipeline (for HW timing slope only)."""
    TA = EC // 256
    KC1 = (2 * D) // 128  # 16 k-chunks for mm1
    KC2 = D // 128        # 8 k-chunks for node matmuls
    MC = D // 128         # 8 m-chunks

    nc = bacc.Bacc("TRN2", target_bir_lowering=False, debug=False, num_devices=C)

    # ---- DRAM I/O ----
    xfull = nc.dram_tensor("xfull", [N, D], F32, kind="ExternalInput").ap()
    eafull = nc.dram_tensor("eafull", [E, D], F32, kind="ExternalInput").ap()
    x_myT = nc.dram_tensor("x_myT", [D, NP], F32R, kind="ExternalInput").ap()
    srcidx = nc.dram_tensor("srcidx", [128, EC // 128], I32, kind="ExternalInput").ap()
    eaidx = nc.dram_tensor("eaidx", [128, EC // 128], I32, kind="ExternalInput").ap()
    gidx = nc.dram_tensor("gidx", [128, NSEG * F2], I32, kind="ExternalInput").ap()
    lidx = nc.dram_tensor("lidx", [128, NSEG * F2], F32, kind="ExternalInput").ap()
    invc = nc.dram_tensor("invc", [128, NSEG], F32, kind="ExternalInput").ap()
    maskv = nc.dram_tensor("maskv", [1, NP], F32R, kind="ExternalInput").ap()
    iota_d = nc.dram_tensor("iota_d", [128, 128], F32, kind="ExternalInput").ap()
    ident_d = nc.dram_tensor("ident_d", [128, 128], F32R, kind="ExternalInput").ap()
    ones_d = nc.dram_tensor("ones_d", [1, 256], F32R, kind="ExternalInput").ap()
    w1a_d = nc.dram_tensor("w1a_d", [2 * D, D], F32R, kind="ExternalInput").ap()
    wb1_d = nc.dram_tensor("wb1_d", [D, D], F32R, kind="ExternalInput").ap()
    w3_d = nc.dram_tensor("w3_d", [D, D], F32R, kind="ExternalInput").ap()
    w2b_d = nc.dram_tensor("w2b_d", [D, D], F32R, kind="ExternalInput").ap()
    b1a_d = nc.dram_tensor("b1a_d", [1, D], F32R, kind="ExternalInput").ap()
    u_d = nc.dram_tensor("u_d", [1, D], F32R, kind="ExternalInput").ap()
    b2a_d = nc.dram_tensor("b2a_d", [128, 8], F32, kind="ExternalInput").ap()
    b2b_d = nc.dram_tensor("b2b_d", [128, 8], F32, kind="ExternalInput").ap()
    out_myT = nc.dram_tensor("out_myT", [D, NP], F32R, kind="ExternalOutput").ap()
    g_dram = nc.dram_tensor("g_scratch", [EC, D], F32R).ap()

    with tile.TileContext(nc) as tc, ExitStack() as ctx:
        cpool = ctx.enter_context(tc.tile_pool(name="consts", bufs=1))
        wpool = ctx.enter_context(tc.tile_pool(name="weights", bufs=KC2 * 3))
        p2 = ctx.enter_context(tc.tile_pool(name="work2", bufs=2))
        p6 = ctx.enter_context(tc.tile_pool(name="work6", bufs=6))
        k1 = ctx.enter_context(tc.tile_pool(name="kslots", bufs=1))
        psum2 = ctx.enter_context(tc.tile_pool(name="psum2", bufs=2, space="PSUM"))

        # ---- constants (gather indices first so phase A starts instantly) --
        ident = cpool.tile([128, 128], F32R, tag="ident")
        nc.sync.dma_start(ident[:], ident_d[:])
        srcidx_sb = cpool.tile([128, EC // 128], I32, tag="srcidx")
        nc.sync.dma_start(srcidx_sb[:], srcidx[:])
        eaidx_sb = cpool.tile([128, EC // 128], I32, tag="eaidx")
        nc.sync.dma_start(eaidx_sb[:], eaidx[:])
        ones_sb = cpool.tile([1, 256], F32R, tag="ones")
        nc.sync.dma_start(ones_sb[:], ones_d[:])
        b1a_sb = cpool.tile([1, D], F32R, tag="b1a")
        nc.sync.dma_start(b1a_sb[:], b1a_d[:])
        iota_sb = cpool.tile([128, 128], F32, tag="iota")
        nc.sync.dma_start(iota_sb[:], iota_d[:])
        u_sb = cpool.tile([1, D], F32R, tag="u")
        nc.sync.dma_start(u_sb[:], u_d[:])
        b2a_sb = cpool.tile([128, 8], F32, tag="b2a")
        nc.sync.dma_start(b2a_sb[:], b2a_d[:])
        b2b_sb = cpool.tile([128, 8], F32, tag="b2b")
        nc.sync.dma_start(b2b_sb[:], b2b_d[:])
        invc_sb = cpool.tile([128, NSEG], F32, tag="invc")
        nc.sync.dma_start(invc_sb[:], invc[:])
        gidx_sb = cpool.tile([128, NSEG * F2], I32, tag="gidx")
        nc.sync.dma_start(gidx_sb[:], gidx[:])
        lidx_sb = cpool.tile([128, NSEG * F2], F32, tag="lidx")
        nc.sync.dma_start(lidx_sb[:], lidx[:])

        for rep in range(reps):
            R = f"r{rep}_" if reps > 1 else ""

            # ---- phase A weights: W1a as 16 k-chunk tiles [128, D] ----
            w1a_sb = []
            for k in range(KC1):
                t = wpool.tile([128, D], F32R, tag="wchunk", name=f"{R}w1a{k}")
                nc.sync.dma_start(t[:], w1a_d[128 * k : 128 * (k + 1), :])
                w1a_sb.append(t)

            # ================= Phase A: edge MLP1 =================
            for i in range(TA):
                xg = []
                eag = []
                for s in range(2):
                    xt = p6.tile([128, D], F32R, tag="gbig", name=f"{R}xg{i}_{s}", bufs=10)
                    nc.gpsimd.indirect_dma_start(
                        out=xt[:],
                        out_offset=None,
                        in_=xfull[:],
                        in_offset=bass.IndirectOffsetOnAxis(
                            ap=srcidx_sb[:, 2 * i + s : 2 * i + s + 1], axis=0
                        ),
                    )
                    xg.append(xt)
                    et = p6.tile([128, D], F32R, tag="eag", name=f"{R}ea{i}_{s}", bufs=5)
                    nc.gpsimd.indirect_dma_start(
                        out=et[:],
                        out_offset=None,
                        in_=eafull[:],
                        in_offset=bass.IndirectOffsetOnAxis(
                            ap=eaidx_sb[:, 2 * i + s : 2 * i + s + 1], axis=0
                        ),
                    )
                    eag.append(et)

                # transpose gathered [e, feat] -> hinT[k] [feat(128), 256 e]
                hinT = [
                    k1.tile([128, 256], F32R, tag=f"hinT{k}", name=f"{R}hinT{i}_{k}")
                    for k in range(KC1)
                ]
                for k in range(KC1):
                    src_list = xg if k < KC2 else eag
                    kk = k if k < KC2 else k - KC2
                    for s in range(2):
                        tp = psum2.tile(
                            [128, 128], F32R, tag="ps1", name=f"{R}tpa{i}_{k}_{s}",
                            bufs=4, padded_shape=[128, 256],
                        )
                        nc.tensor.transpose(
                            tp[:], src_list[s][:, 128 * kk : 128 * (kk + 1)], ident[:]
                        )
                        nc.vector.tensor_copy(hinT[k][:, 128 * s : 128 * (s + 1)], tp[:])

                # mm1: h1[e,:] = sum_k hinT[k].T @ W1a[k] + ones.T @ b1a
                for s in range(2):
                    ph = psum2.tile([128, D], F32, tag="big", name=f"{R}ph{i}_{s}")
                    for k in range(KC1):
                        lt = hinT[k][:, 128 * s : 128 * (s + 1)]
                        for h in range(2):
                            nc.tensor.matmul(
                                ph[:, 512 * h : 512 * (h + 1)],
                                lt,
                                w1a_sb[k][:, 512 * h : 512 * (h + 1)],
                                start=(k == 0),
                                stop=False,
                            )
                    for h in range(2):
                        nc.tensor.matmul(
                            ph[:, 512 * h : 512 * (h + 1)],
                            ones_sb[0:1, 128 * s : 128 * s + 128],
                            b1a_sb[0:1, 512 * h : 512 * (h + 1)],
                            start=False,
                            stop=True,
                        )
                    gsb = p2.tile([128, D], F32R, tag="gsbrm", name=f"{R}gsb{i}_{s}")
                    nc.scalar.activation(gsb[:], ph[:], AF.Relu)
                    nc.sync.dma_start(
                        g_dram[256 * i + 128 * s : 256 * i + 128 * (s + 1), :], gsb[:]
                    )

            # ---- phase B weights ----
            wb1_sb, w3_sb, w2b_sb = [], [], []
            for wd, lst, nm in (
                (wb1_d, wb1_sb, "wb1"),
                (w3_d, w3_sb, "w3"),
                (w2b_d, w2b_sb, "w2b"),
            ):
                for k in range(KC2):
                    t = wpool.tile([128, D], F32R, tag="wchunk", name=f"{R}{nm}{k}")
                    nc.sync.dma_start(t[:], wd[128 * k : 128 * (k + 1), :])
                    lst.append(t)

            # ================= Phase B: segment mean + MLP2 =================
            for t2 in range(NT2):
                rmT = [
                    k1.tile([128, 256], F32R, tag=f"hinT{k + 8}", name=f"{R}rmT{t2}_{k}")
                    for k in range(KC2)
                ]
                for h in range(2):
                    q = 2 * t2 + h
                    # segment sums for node tile q (128 packed node slots)
                    pr = psum2.tile([128, D], F32, tag="big", name=f"{R}pr{q}")
                    for j in range(F2):
                        ge = p6.tile(
                            [128, D], F32R, tag="gbig", name=f"{R}ge{q}_{j}", bufs=10
                        )
                        nc.gpsimd.indirect_dma_start(
                            out=ge[:],
                            out_offset=None,
                            in_=g_dram[:],
                            in_offset=bass.IndirectOffsetOnAxis(
                                ap=gidx_sb[:, F2 * q + j : F2 * q + j + 1], axis=0
                            ),
                        )
                        S = p6.tile(
                            [128, 128], F32R, tag="S", name=f"{R}S{q}_{j}", bufs=8
                        )
                        nc.vector.tensor_tensor(
                            out=S[:],
                            in0=lidx_sb[:, F2 * q + j : F2 * q + j + 1].to_broadcast(
                                [128, 128]
                            ),
                            in1=iota_sb[:],
                            op=OP.is_equal,
                        )
                        for nh in range(2):
                            nc.tensor.matmul(
                                pr[:, 512 * nh : 512 * (nh + 1)],
                                S[:],
                                ge[:, 512 * nh : 512 * (nh + 1)],
                                start=(j == 0),
                                stop=(j == F2 - 1),
                            )
                    # rmean = sums * invc ; transpose into rmT[k][:, 128h:...]
                    rm = p2.tile([128, D], F32R, tag="gsbrm", name=f"{R}rm{q}")
                    nc.scalar.mul(rm[:], pr[:], invc_sb[:, q : q + 1])
                    for k in range(KC2):
                        tp = psum2.tile(
                            [128, 128], F32R, tag="ps1", name=f"{R}tpr{q}_{k}",
                            bufs=4, padded_shape=[128, 256],
                        )
                        nc.tensor.transpose(
                            tp[:], rm[:, 128 * k : 128 * (k + 1)], ident[:]
                        )
                        nc.vector.tensor_copy(rmT[k][:, 128 * h : 128 * (h + 1)], tp[:])

                # x tile: direct strided loads from host-transposed x
                xT = [
                    k1.tile([128, 256], F32R, tag=f"xT{k}", name=f"{R}xT{t2}_{k}")
                    for k in range(KC2)
                ]
                for k in range(KC2):
                    nc.sync.dma_start(
                        xT[k][:],
                        x_myT[128 * k : 128 * (k + 1), 256 * t2 : 256 * (t2 + 1)],
                    )

                msk = p2.tile([1, 256], F32R, tag="msk", name=f"{R}msk{t2}")
                nc.sync.dma_start(msk[:], maskv[0:1, 256 * t2 : 256 * (t2 + 1)])

                # mm2a: o1T[m] = relu(sum_k B1[k,m].T@xT[k] + W3[k,m].T@rmT[k]
                #                     + u[m] x mask + b2a[m])
                o1T = []
                for m in range(MC):
                    pb = psum2.tile(
                        [128, 256], F32, tag="ps1", name=f"{R}pa{t2}_{m}", bufs=4
                    )
                    for k in range(KC2):
                        nc.tensor.matmul(
                            pb[:],
                            wb1_sb[k][:, 128 * m : 128 * (m + 1)],
                            xT[k][:],
                            start=(k == 0),
                            stop=False,
                        )
                    for k in range(KC2):
                        nc.tensor.matmul(
                            pb[:],
                            w3_sb[k][:, 128 * m : 128 * (m + 1)],
                            rmT[k][:],
                            start=False,
                            stop=False,
                        )
                    nc.tensor.matmul(
                        pb[:],
                        u_sb[0:1, 128 * m : 128 * (m + 1)],
                        msk[:],
                        start=False,
                        stop=True,
                    )
                    ot = k1.tile([128, 256], F32R, tag=f"hinT{m}", name=f"{R}o1T{t2}_{m}")
                    nc.scalar.activation(
                        ot[:], pb[:], AF.Relu, bias=b2a_sb[:, m : m + 1]
                    )
                    o1T.append(ot)

                # mm2b: o2T[m] = sum_k W2b[k,m].T @ o1T[k] + b2b[m]; store
                for m in range(MC):
                    pb = psum2.tile(
                        [128, 256], F32, tag="ps1", name=f"{R}pb{t2}_{m}", bufs=4
                    )
                    for k in range(KC2):
                        nc.tensor.matmul(
                            pb[:],
                            w2b_sb[k][:, 128 * m : 128 * (m + 1)],
                            o1T[k][:],
                            start=(k == 0),
                            stop=(k == KC2 - 1),
                        )
                    ot = k1.tile(
                        [128, 256], F32R, tag=f"hinT{m + 8}", name=f"{R}o2T{t2}_{m}"
                    )
                    nc.scalar.activation(
                        ot[:], pb[:], AF.Identity, bias=b2b_sb[:, m : m + 1]
                    )
                    nc.sync.dma_start(
                        out_myT[128 * m : 128 * (m + 1), 256 * t2 : 256 * (t2 + 1)],
                        ot[:],
                    )

    nc.compile()
    return nc


def _get_program(EC, F2):
    key = (EC, F2)
    if key not in _PROGRAM_CACHE:
        _PROGRAM_CACHE[key] = _build_program(EC, F2)
    return _PROGRAM_CACHE[key]


def _pad_to(a, n, fill):
    out = np.full((n,) + a.shape[1:], fill, dtype=a.dtype)
    out[: a.shape[0]] = a
    return out


def _pack_nodes(deg):
    """Bin-pack NPC nodes (weight = degree) into NSEG tiles of <=128 slots,
    balancing total degree. Returns (order, tile_load): order[pos] = local
    node id or -1 for an empty slot, where pos = 128*q + p."""
    nodes = np.argsort(-deg, kind="stable")
    heap = [(0, 0, q) for q in range(NSEG)]  # (load, used, q)
    heapq.heapify(heap)
    order = np.full(NP, -1, np.int64)
    load = np.zeros(NSEG, np.int64)
    for n in nodes:
        while True:
            l, u, q = heapq.heappop(heap)
            if u < 128:
                break
        order[128 * q + u] = n
        load[q] = l + int(deg[n])
        heapq.heappush(heap, (load[q], u + 1, q))
    return order, load


def _make_in_maps(x, edge_index, edge_attr, W1a, b1a, W1b, b1b, W2a, b2a, W2b, b2b):
    """Host preprocessing. Returns (EC, F2, in_maps, orders)."""
    x = np.ascontiguousarray(np.asarray(x, np.float32))
    edge_attr = np.ascontiguousarray(np.asarray(edge_attr, np.float32))
    ei = np.asarray(edge_index)
    row, col = ei[0].astype(np.int64), ei[1].astype(np.int64)

    perm = np.argsort(col, kind="stable")
    col_s = col[perm]
    row_s = row[perm]
    core_bounds = np.searchsorted(col_s, NPC * np.arange(C + 1))
    core_cnt = np.diff(core_bounds)
    EC = max(256, int(np.ceil(core_cnt.max() / 256)) * 256)

    counts = np.bincount(col, minlength=N)

    orders = []
    F2 = 1
    for c in range(C):
        deg = counts[NPC * c : NPC * (c + 1)]
        order, load = _pack_nodes(deg)
        orders.append(order)
        F2 = max(F2, int(np.ceil(load.max() / 128)))

    # ---- fold weights on host (float64 for accuracy) ----
    W1a = np.ascontiguousarray(np.asarray(W1a, np.float32))
    B1 = np.ascontiguousarray(np.asarray(W2a, np.float64)[:D])
    B2 = np.ascontiguousarray(np.asarray(W2a, np.float64)[D:])
    W3 = (np.asarray(W1b, np.float64) @ B2).astype(np.float32)
    u = (np.asarray(b1b, np.float64) @ B2).astype(np.float32)
    B1 = B1.astype(np.float32)
    iota = np.broadcast_to(np.arange(128, dtype=np.float32), (128, 128)).copy()

    in_maps = []
    for c in range(C):
        s, e = core_bounds[c], core_bounds[c + 1]
        lo = NPC * c
        src_c = _pad_to(row_s[s:e].astype(np.int32), EC, 0)
        ea_c = _pad_to(perm[s:e].astype(np.int32), EC, 0)
        cnt_loc = counts[lo : lo + NPC]
        starts = np.zeros(NPC + 1, np.int64)
        np.cumsum(cnt_loc, out=starts[1:])
        order = orders[c]
        gi = np.zeros((NSEG, F2 * 128), np.int32)
        li = np.full((NSEG, F2 * 128), 300.0, np.float32)
        for q in range(NSEG):
            pos = 0
            for p in range(128):
                n = order[128 * q + p]
                if n < 0:
                    continue
                a, b = starts[n], starts[n + 1]
                k = b - a
                gi[q, pos : pos + k] = np.arange(a, b, dtype=np.int32)
                li[q, pos : pos + k] = float(p)
                pos += k
            assert pos <= F2 * 128
        ordc = np.maximum(order, 0)
        valid = order >= 0
        cnt_c = np.where(valid, cnt_loc[ordc], 0).astype(np.float32)
        invc_c = (1.0 / np.maximum(cnt_c, 1.0)).astype(np.float32)
        mask_c = ((cnt_c > 0) & valid).astype(np.float32)
        x_c = np.where(valid[:, None], x[lo + ordc], 0.0).astype(np.float32)

        in_maps.append(
            {
                "xfull": x,
                "eafull": edge_attr,
                "x_myT": np.ascontiguousarray(x_c.T),
                "srcidx": src_c.reshape(EC // 128, 128).T.copy(),
                "eaidx": ea_c.reshape(EC // 128, 128).T.copy(),
                "gidx": gi.reshape(NSEG * F2, 128).T.copy(),
                "lidx": li.reshape(NSEG * F2, 128).T.copy(),
                "invc": invc_c.reshape(NSEG, 128).T.copy(),
                "maskv": mask_c.reshape(1, NP),
                "iota_d": iota,
                "ident_d": np.eye(128, dtype=np.float32),
                "ones_d": np.ones((1, 256), np.float32),
                "w1a_d": W1a,
                "wb1_d": B1,
                "w3_d": W3,
                "w2b_d": np.ascontiguousarray(np.asarray(W2b, np.float32)),
                "b1a_d": np.asarray(b1a, np.float32).reshape(1, D),
                "u_d": u.reshape(1, D),
                "b2a_d": np.asarray(b2a, np.float32).reshape(8, 128).T.copy(),
                "b2b_d": np.asarray(b2b, np.float32).reshape(8, 128).T.copy(),
            }
        )
    return EC, F2, in_maps, orders


def kernel(x, edge_index, edge_attr, W1a, b1a, W1b, b1b, W2a, b2a, W2b, b2b):
    global _LAST_IN_MAPS
    EC, F2, in_maps, orders = _make_in_maps(
        x, edge_index, edge_attr, W1a, b1a, W1b, b1b, W2a, b2a, W2b, b2b
    )
    nc = _get_program(EC, F2)
    _LAST_IN_MAPS = in_maps
    res = run_bass_kernel_spmd(nc, in_maps, core_ids=list(range(C)))
    out = np.empty((N, D), np.float32)
    for c in range(C):
        o = np.asarray(res.results[c]["out_myT"]).T  # [NP, D]
        order = orders[c]
        valid = order >= 0
        out[NPC * c + order[valid]] = o[valid]
    return np.ascontiguousarray(out)
